# revision 1
# baseline (speedup 1.0000x reference)
"""Trainium2 Bass kernel for nn_Decoder (GRU decoder with clipped-delta
position integration).

Strategy
--------
Data-parallel over the batch N=16384: 8 cores x 2048 rows. Everything on-chip
per core runs in a *transposed* (feature-major) layout so the recurrent
matmul W_hh @ h streams h as the moving operand with weights stationary and
no per-step transposes are needed:

  h      [HID=256, 2048]  as SBUF [128, 2, 512] x4 chunks  (K-tile, batch)
  gates  [768, chunk=512] in PSUM, gate-major
  x_c    [8, 512] per chunk: rows 0-1 = prev delta (dx, dy), rows 2-6 = ctx.

Per step, per 512-column chunk:
  P1[mt<4] = W_hh[0:512] @ h + W_ih_aug @ x     (rz preact, PSUM; K=7 tail)
  P2[2]    = W_ih_aug @ x                       (i_n contribution)
  P3[2]    = W_hh[512:768] @ h                  (h_n contribution)
  r,z = sigmoid(P1 + b_rz)  -- biases ride the free per-partition ACT bias
  npre = (P2 + b_ihn) + r*(P3 + b_hhn)          -- biases via STT scalar APs
  n = tanh(npre); h = (1-z)*n + z*h on GPSIMD (Pool has no subtract/STT,
  so 1-z = (z*-1)+1 via tensor_scalar). h keeps an fp32 master copy plus a
  bf16 shadow (refreshed by one Pool copy per chunk) that feeds the PE.
  delta = W_out @ h_new  -> bias-add eviction into a spread [128,512] tile
                            (chunk c pair at partitions 32c, 32c+1).
Step-level clip: pair-sum matmul gives u' = -0.5*|d|^2/max_step^2 spread over
the same partitions; quake-seed + 2 Newton iterations on the DVE give
rsqrt(u) (no ACT table switch, all lanes busy); delta_clipped feeds pos
(+= on GPSIMD) and is written back into the x tiles by 32-aligned
DVE/Pool muls (no DMAs on the recurrence path). Output: 4 small DMAs/step.

Matmul operands are bf16 (1 col/cycle on the PE, FWL weight loads; fp32
runs at 1/4 rate and float32r trips walrus sync-wait limits); PSUM
accumulation is fp32 and the norm matmul stays fp32 for precision.
Measured on trn2 (8 axon cores): ~19 us/step -> ~1.8 ms for T=96,
absmax error ~4.6e-3 relative to absmax(reference).
"""

import sys

for _p in ("/opt/trn_rl_repo",):
    if _p not in sys.path:
        sys.path.insert(0, _p)

import numpy as np

import concourse.bass as bass
import concourse.tile as tile
from concourse.bacc import Bacc
from concourse import mybir
from concourse.bass_utils import run_bass_kernel_spmd

F32 = mybir.dt.float32
F32R = mybir.dt.float32r
BF16 = mybir.dt.bfloat16
I32 = mybir.dt.int32
AF = mybir.ActivationFunctionType
OP = mybir.AluOpType

HID = 256
CTX_DIM = 5
V_MAX = 10.1415
DT = 0.093
MS = V_MAX * DT  # max_step
N_CORES = 8
MAGIC = 0x5F3759DF - 0x400000  # quake magic adjusted for input u' = -0.5*u


def build_module(T: int, nloc: int, unroll: int = 0):
    """Trace the Bass/Tile module for one core (nloc batch columns)."""
    CH = nloc // 512  # column chunks of 512
    assert nloc % 512 == 0

    nc = Bacc()

    # ---- DRAM I/O ----
    h0_d = nc.dram_tensor("h0", [2, 128, nloc], F32, kind="ExternalInput")
    h0b_d = nc.dram_tensor("h0b", [2, 128, nloc], BF16, kind="ExternalInput")
    x0_d = nc.dram_tensor("x0i", [8, nloc], BF16, kind="ExternalInput")
    pos0_d = nc.dram_tensor("pos0", [128, 512], F32, kind="ExternalInput")
    wh_d = nc.dram_tensor("wh", [2, 128, 768], BF16, kind="ExternalInput")
    wt_d = nc.dram_tensor("wt", [8, 6, 128], BF16, kind="ExternalInput")
    wo_d = nc.dram_tensor("wo", [2, 128, 2], BF16, kind="ExternalInput")
    wd2_d = nc.dram_tensor("wd2", [128, 128], F32, kind="ExternalInput")
    bv_d = nc.dram_tensor("bv", [128, 8], F32, kind="ExternalInput")
    bpk_d = nc.dram_tensor("bpk", [2, 1], F32, kind="ExternalInput")
    out_d = nc.dram_tensor("out", [T, 2 * CH, 512], F32, kind="ExternalOutput")

    with tile.TileContext(nc) as tc:
        import contextlib

        ctx = contextlib.ExitStack()
        with ctx:
            singles = ctx.enter_context(tc.tile_pool(name="singles", bufs=1))
            h_c = []
            x_c = []
            hb_c = []
            for c in range(CH):
                h_c.append(singles.tile([128, 2, 512], F32, tag=f"h{c}", name=f"h{c}"))
                x_c.append(singles.tile([8, 512], BF16, tag=f"x{c}", name=f"x{c}"))
                hb_c.append(singles.tile([128, 2, 512], BF16, tag=f"hb{c}", name=f"hb{c}"))
            pos = singles.tile([128, 512], F32, tag="pos", name="pos")
            dbtw = singles.tile([128, 512], F32, tag="dbtw", name="dbtw")
            wh = singles.tile([128, 2, 768], BF16, tag="wh", name="wh")
            wt = singles.tile([8, 6, 128], BF16, tag="wt", name="wt")
            wo = singles.tile([128, 2, 2], BF16, tag="wo", name="wo")
            wd2 = singles.tile([128, 128], F32, tag="wd2", name="wd2")
            bv = singles.tile([128, 8], F32, tag="bv", name="bv")
            bpk = singles.tile([2, 1], F32, tag="bpk", name="bpk")
            nc.vector.memset(dbtw, 0.0)

            # initial loads
            for c in range(CH):
                cs = slice(c * 512, (c + 1) * 512)
                nc.sync.dma_start(
                    out=h_c[c],
                    in_=h0_d[:, :, :].transpose([1, 0, 2])[:, :, cs])
                nc.sync.dma_start(
                    out=hb_c[c],
                    in_=h0b_d[:, :, :].transpose([1, 0, 2])[:, :, cs])
                nc.sync.dma_start(out=x_c[c], in_=x0_d[:, :][:, cs])
            nc.sync.dma_start(out=pos, in_=pos0_d[:, :])
            nc.sync.dma_start(out=wh, in_=wh_d[:, :, :].transpose([1, 0, 2]))
            nc.sync.dma_start(out=wt, in_=wt_d[:, :, :])
            nc.sync.dma_start(out=wo, in_=wo_d[:, :, :].transpose([1, 0, 2]))
            nc.sync.dma_start(out=wd2, in_=wd2_d[:, :])
            nc.sync.dma_start(out=bv, in_=bv_d[:, :])
            nc.sync.dma_start(out=bpk, in_=bpk_d[:, :])

            # pools
            pp1 = ctx.enter_context(tc.tile_pool(name="pp1", bufs=4, space="PSUM"))
            pp2 = ctx.enter_context(tc.tile_pool(name="pp2", bufs=2, space="PSUM"))
            pp3 = ctx.enter_context(tc.tile_pool(name="pp3", bufs=1, space="PSUM"))
            ppd = ctx.enter_context(tc.tile_pool(name="ppd", bufs=1, space="PSUM"))
            sb = ctx.enter_context(tc.tile_pool(name="sb", bufs=3))
            sbs = ctx.enter_context(tc.tile_pool(name="sbs", bufs=3))

            def step(t_idx):
                for c in range(CH):
                    hc = h_c[c]
                    hb = hb_c[c]
                    xc = x_c[c]
                    # --- P1: rz preactivations, 4 M-tiles ---
                    rzs = sb.tile([128, 4, 512], F32, tag="rzs", name="rzs")
                    for mt in range(4):
                        p1 = pp1.tile([128, 512], F32, tag="p1", name="p1")
                        ms_ = slice(mt * 128, (mt + 1) * 128)
                        nc.tensor.matmul(
                            p1, wh[:, 0, ms_],
                            hb[:, 0, :], start=True, stop=False)
                        nc.tensor.matmul(
                            p1, wh[:, 1, ms_],
                            hb[:, 1, :], start=False, stop=False)
                        nc.tensor.matmul(
                            p1, wt[0:7, mt, :],
                            xc[0:7, :],
                            start=False, stop=True)
                        nc.scalar.activation(
                            rzs[:, mt, :], p1, AF.Sigmoid,
                            bias=bv[:, mt:mt + 1])
                    # --- P2: i_n, P3: h_n ---
                    p2s, p3s = [], []
                    for i in range(2):
                        p2 = pp2.tile([128, 512], F32, tag="p2", name="p2")
                        nc.tensor.matmul(
                            p2, wt[0:7, 4 + i, :],
                            xc[0:7, :],
                            start=True, stop=True)
                        p2s.append(p2)
                    for i in range(2):
                        p3 = pp3.tile([128, 512], F32, tag="p3", name="p3")
                        ms_ = slice(512 + i * 128, 512 + (i + 1) * 128)
                        nc.tensor.matmul(
                            p3, wh[:, 0, ms_],
                            hb[:, 0, :], start=True, stop=False)
                        nc.tensor.matmul(
                            p3, wh[:, 1, ms_],
                            hb[:, 1, :], start=False, stop=True)
                        p3s.append(p3)
                    # --- npre = (P2 + b_ihn) + r*(P3 + b_hhn); n = tanh ---
                    npre = sb.tile([128, 2, 512], F32, tag="npre", name="npre")
                    for i in range(2):
                        t1 = sbs.tile([128, 512], F32, tag="t1", name="t1")
                        nc.vector.scalar_tensor_tensor(
                            t1, p3s[i], bv[:, 6 + i:7 + i], rzs[:, i, :],
                            op0=OP.add, op1=OP.mult)
                        nc.vector.scalar_tensor_tensor(
                            npre[:, i, :], p2s[i], bv[:, 4 + i:5 + i], t1,
                            op0=OP.add, op1=OP.add)
                    n_t = sb.tile([128, 2, 512], F32, tag="n", name="n")
                    for i in range(2):
                        nc.scalar.activation(
                            n_t[:, i, :], npre[:, i, :], AF.Tanh)
                    # --- h = (1-z)*n + z*h  (Pool only: TS + TT ops) ---
                    for kt in range(2):
                        eng = nc.gpsimd
                        zc = sbs.tile([128, 512], F32, tag=f"zc{kt}", name=f"zc{kt}")
                        eng.tensor_scalar(
                            zc, rzs[:, 2 + kt, :], -1.0, 1.0,
                            op0=OP.mult, op1=OP.add)
                        d_t = sbs.tile([128, 512], F32, tag=f"d{kt}", name=f"d{kt}")
                        eng.tensor_mul(d_t, zc, n_t[:, kt, :])
                        u_t = sbs.tile([128, 512], F32, tag=f"u{kt}", name=f"u{kt}")
                        eng.tensor_mul(u_t, rzs[:, 2 + kt, :], hc[:, kt, :])
                        eng.tensor_add(hc[:, kt, :], d_t, u_t)
                        nc.gpsimd.tensor_copy(hb[:, kt, :], hc[:, kt, :])
                    # --- delta = W_out @ h_new, spread eviction ---
                    pd = ppd.tile([2, 512], F32, tag="pdu", name="pdu")
                    nc.tensor.matmul(pd, wo[:, 0, :],
                                     hb[:, 0, :],
                                     start=True, stop=False)
                    nc.tensor.matmul(pd, wo[:, 1, :],
                                     hb[:, 1, :],
                                     start=False, stop=True)
                    nc.vector.tensor_scalar(
                        dbtw[32 * c:32 * c + 2, :], pd, bpk[0:2, :], None,
                        op0=OP.add)

                # ---- clip: s = min(MS/||delta||, 1), spread [128, 512] ----
                sqv = sbs.tile([128, 512], F32, tag="sqv", name="sqv")
                nc.gpsimd.tensor_mul(sqv, dbtw, dbtw)
                pu = ppd.tile([128, 512], F32, tag="pdu", name="pu")
                nc.tensor.matmul(pu, wd2, sqv, start=True, stop=True)
                s1i = sbs.tile([128, 512], I32, tag="s1i", name="s1i")
                nc.vector.tensor_scalar(
                    s1i, pu.bitcast(I32), 1, 0x3FFFFFFF,
                    op0=OP.logical_shift_right, op1=OP.bitwise_and)
                y0i = sbs.tile([128, 512], I32, tag="y0i", name="y0i")
                nc.vector.tensor_scalar(
                    y0i, s1i, MAGIC, -1, op0=OP.subtract, op1=OP.mult)
                y = y0i.bitcast(F32)
                ys = []
                for it in range(2):
                    m_t = sbs.tile([128, 512], F32, tag=f"m{it}", name=f"m{it}")
                    nc.vector.tensor_mul(m_t, y, y)
                    m2_t = sbs.tile([128, 512], F32, tag=f"m2{it}", name=f"m2{it}")
                    nc.vector.tensor_mul(m2_t, m_t, pu)
                    y2_t = sbs.tile([128, 512], F32, tag=f"y2{it}", name=f"y2{it}")
                    nc.vector.scalar_tensor_tensor(
                        y2_t, m2_t, 1.5, y, op0=OP.add, op1=OP.mult)
                    y = y2_t
                    ys.append(y)
                    if it == 0:
                        # x feedback tolerates 1-Newton precision (it is
                        # bf16-rounded anyway) -> unblock next step early
                        smin1 = sbs.tile([128, 512], F32, tag="smin1",
                                         name="smin1")
                        nc.vector.tensor_scalar(
                            smin1, y, 1.0, None, op0=OP.min)
                        for c in range(CH):
                            eng = nc.vector if c % 2 == 0 else nc.gpsimd
                            eng.tensor_mul(
                                x_c[c][0:2, :], smin1[32 * c:32 * c + 2, :],
                                dbtw[32 * c:32 * c + 2, :])
                # pos/output keep the 2-Newton value
                smin = sbs.tile([128, 512], F32, tag="smin", name="smin")
                nc.gpsimd.tensor_scalar(smin, y, 1.0, None, op0=OP.min)
                dct = sbs.tile([128, 512], F32, tag="dct", name="dct")
                nc.gpsimd.tensor_mul(dct, smin, dbtw)
                nc.gpsimd.tensor_add(pos, pos, dct)
                for c in range(CH):
                    nc.sync.dma_start(
                        out=out_d[t_idx, 2 * c:2 * c + 2, :],
                        in_=pos[32 * c:32 * c + 2, :])

            if unroll <= 0:
                for t in range(T):
                    step(t)
            else:
                assert T % unroll == 0
                n_iter = T // unroll
                with tc.For_i(0, n_iter * unroll, unroll) as iv:
                    for j in range(unroll):
                        step(iv + j)

    nc.finalize()
    return nc


# ---------------- host side ----------------

_module_cache: dict = {}


def _get_module(T: int, nloc: int, unroll: int):
    key = (T, nloc, unroll)
    if key not in _module_cache:
        _module_cache[key] = build_module(T, nloc, unroll)
    return _module_cache[key]


def _host_prep(inputs, nloc):
    """Build per-core in_maps from full inputs."""
    N = inputs["init_h"].shape[0]
    n_sh = N // N_CORES
    CH = nloc // 512
    W_ih = np.asarray(inputs["W_ih"], np.float32)
    W_hh = np.asarray(inputs["W_hh"], np.float32)
    b_ih = np.asarray(inputs["b_ih"], np.float32)
    b_hh = np.asarray(inputs["b_hh"], np.float32)
    W_out = np.asarray(inputs["W_out"], np.float32)
    b_out = np.asarray(inputs["b_out"], np.float32)

    import ml_dtypes
    bf16 = ml_dtypes.bfloat16
    wh = np.ascontiguousarray(W_hh.T.reshape(2, 128, 768)).astype(bf16)
    wo = np.ascontiguousarray(W_out.T.reshape(2, 128, 2)).astype(bf16)

    # K=7 input tails: rows 0-1 = delta cols of W_ih, rows 2-6 = ctx cols
    wt = np.zeros((8, 6, 128), bf16)
    for mt in range(6):
        if mt < 4:
            rows = slice(mt * 128, (mt + 1) * 128)
        else:
            rows = slice(512 + (mt - 4) * 128, 512 + (mt - 3) * 128)
        wt[0:7, mt, :] = W_ih[rows, :].T.astype(bf16)

    # biases: cols 0-3 = (b_ih+b_hh) rz tiles, 4-5 = b_ih n, 6-7 = b_hh n
    bv = np.zeros((128, 8), np.float32)
    for mt in range(4):
        bv[:, mt] = (b_ih + b_hh)[mt * 128:(mt + 1) * 128]
    for i in range(2):
        bv[:, 4 + i] = b_ih[512 + i * 128:512 + (i + 1) * 128]
        bv[:, 6 + i] = b_hh[512 + i * 128:512 + (i + 1) * 128]

    wd2 = np.zeros((128, 128), np.float32)
    for c in range(CH):
        for i in range(2):
            for j in range(2):
                wd2[32 * c + i, 32 * c + j] = -0.5 / (MS * MS)

    bpk = np.asarray(b_out, np.float32).reshape(2, 1)

    init_h = np.asarray(inputs["init_h"], np.float32)
    ctx_in = np.asarray(inputs["ctx"], np.float32)
    x0 = np.asarray(inputs["x0"], np.float32)
    y0 = np.asarray(inputs["y0"], np.float32)

    in_maps = []
    for core in range(N_CORES):
        sl = slice(core * n_sh, (core + 1) * n_sh)
        h0 = np.ascontiguousarray(init_h[sl].T.reshape(2, 128, nloc))
        h0b = h0.astype(bf16)
        x0i = np.zeros((8, nloc), bf16)
        x0i[2:7] = ctx_in[sl].T.astype(bf16)
        pos0 = np.zeros((128, 512), np.float32)
        for c in range(CH):
            pos0[32 * c + 0] = x0[sl].reshape(CH, 512)[c]
            pos0[32 * c + 1] = y0[sl].reshape(CH, 512)[c]
        in_maps.append({
            "h0": h0, "h0b": h0b, "x0i": x0i, "pos0": pos0, "wh": wh,
            "wt": wt, "wo": wo, "wd2": wd2, "bv": bv, "bpk": bpk,
        })
    return in_maps


def _host_unpack(results, T, nloc):
    CH = nloc // 512
    outs = []
    for r in results:
        arr = r["out"]  # [T, 2CH, 512] rows 2c+coord
        a = arr.reshape(T, CH, 2, 512).transpose(1, 3, 0, 2)  # ch, s, T, 2
        outs.append(a.reshape(nloc, T, 2))
    return np.concatenate(outs, axis=0)


def kernel(**inputs) -> np.ndarray:
    T = int(inputs["T"])
    N = inputs["init_h"].shape[0]
    nloc = N // N_CORES
    unroll = 0
    nc = _get_module(T, nloc, unroll)
    in_maps = _host_prep(inputs, nloc)
    res = run_bass_kernel_spmd(nc, in_maps, core_ids=list(range(N_CORES)))
    return _host_unpack(res.results, T, nloc)



# revision 2
# speedup vs baseline: 204.5443x; 204.5443x over previous
"""Trainium2 Bass kernel for nn_Decoder (GRU decoder with clipped-delta
position integration).

Strategy
--------
Data-parallel over the batch N=16384: 8 cores x 2048 rows. Everything on-chip
per core runs in a *transposed* (feature-major) layout so the recurrent
matmul W_hh @ h streams h as the moving operand with weights stationary and
no per-step transposes are needed:

  h      [HID=256, 2048]  as SBUF [128, 2, 512] x4 chunks  (K-tile, batch)
  gates  [768, chunk=512] in PSUM, gate-major
  x_c    [8, 512] per chunk: rows 0-1 = prev delta (dx, dy), rows 2-6 = ctx.

Per step, per 512-column chunk:
  P1[mt<4] = W_hh[0:512] @ h + W_ih_aug @ x     (rz preact, PSUM; K=7 tail)
  P2[2]    = W_ih_aug @ x                       (i_n contribution)
  P3[2]    = W_hh[512:768] @ h                  (h_n contribution)
  r,z = sigmoid(P1 + b_rz)  -- biases ride the free per-partition ACT bias
  npre = (P2 + b_ihn) + r*(P3 + b_hhn)          -- biases via STT scalar APs
  n = tanh(npre); h = (1-z)*n + z*h on GPSIMD (Pool has no subtract/STT,
  so 1-z = (z*-1)+1 via tensor_scalar). h keeps an fp32 master copy plus a
  bf16 shadow (refreshed by one Pool copy per chunk) that feeds the PE.
  delta = W_out @ h_new  -> bias-add eviction into a spread [128,512] tile
                            (chunk c pair at partitions 32c, 32c+1).
Step-level clip: pair-sum matmul gives u' = -0.5*|d|^2/max_step^2 spread over
the same partitions; quake-seed + 2 Newton iterations on the DVE give
rsqrt(u) (no ACT table switch, all lanes busy); delta_clipped feeds pos
(+= on GPSIMD) and is written back into the x tiles by 32-aligned
DVE/Pool muls (no DMAs on the recurrence path).

The T loop is a hardware For_i loop (unrolled by `unroll` steps per
iteration) so the program size is O(1) in T; outputs are written per step
as fp16 into a [t_alloc, 8, 512] DRAM tensor (4 small DMAs/step).
`repeats` wraps the T loop in an outer hardware loop for timing runs
(state just keeps evolving; used only by test.py to amplify the per-step
signal above the remote-dispatch noise floor).

Matmul operands are bf16 (1 col/cycle on the PE, FWL weight loads; fp32
runs at 1/4 rate and float32r trips walrus sync-wait limits); PSUM
accumulation is fp32 and the norm matmul stays fp32 for precision.
"""

import sys

for _p in ("/opt/trn_rl_repo",):
    if _p not in sys.path:
        sys.path.insert(0, _p)

import numpy as np

import concourse.bass as bass
import concourse.tile as tile
from concourse.bacc import Bacc
from concourse import mybir
from concourse.bass_utils import run_bass_kernel_spmd

F32 = mybir.dt.float32
F16 = mybir.dt.float16
BF16 = mybir.dt.bfloat16
I32 = mybir.dt.int32
AF = mybir.ActivationFunctionType
OP = mybir.AluOpType

HID = 256
CTX_DIM = 5
V_MAX = 10.1415
DT = 0.093
MS = V_MAX * DT  # max_step
N_CORES = 8
MAGIC = 0x5F3759DF - 0x400000  # quake magic adjusted for input u' = -0.5*u


def build_module(T: int, nloc: int, unroll: int = 2, t_alloc: int | None = None,
                 repeats: int = 1):
    """Trace the Bass/Tile module for one core (nloc batch columns)."""
    CH = nloc // 512  # column chunks of 512
    assert nloc % 512 == 0
    if t_alloc is None:
        t_alloc = T

    nc = Bacc()

    # ---- DRAM I/O ----
    h0_d = nc.dram_tensor("h0", [2, 128, nloc], F32, kind="ExternalInput")
    h0b_d = nc.dram_tensor("h0b", [2, 128, nloc], BF16, kind="ExternalInput")
    x0_d = nc.dram_tensor("x0i", [8, nloc], BF16, kind="ExternalInput")
    pos0_d = nc.dram_tensor("pos0", [128, 512], F32, kind="ExternalInput")
    wh_d = nc.dram_tensor("wh", [2, 128, 768], BF16, kind="ExternalInput")
    wt_d = nc.dram_tensor("wt", [8, 6, 128], BF16, kind="ExternalInput")
    wo_d = nc.dram_tensor("wo", [2, 128, 2], BF16, kind="ExternalInput")
    wd2_d = nc.dram_tensor("wd2", [128, 128], F32, kind="ExternalInput")
    bv_d = nc.dram_tensor("bv", [128, 8], F32, kind="ExternalInput")
    bpk_d = nc.dram_tensor("bpk", [2, 1], F32, kind="ExternalInput")
    out_d = nc.dram_tensor("out", [t_alloc, 2 * CH, 512], F16,
                           kind="ExternalOutput")

    with tile.TileContext(nc) as tc:
        import contextlib

        ctx = contextlib.ExitStack()
        with ctx:
            singles = ctx.enter_context(tc.tile_pool(name="singles", bufs=1))
            h_c = []
            x_c = []
            hb_c = []
            for c in range(CH):
                h_c.append(singles.tile([128, 2, 512], F32, tag=f"h{c}", name=f"h{c}"))
                x_c.append(singles.tile([8, 512], BF16, tag=f"x{c}", name=f"x{c}"))
                hb_c.append(singles.tile([128, 2, 512], BF16, tag=f"hb{c}", name=f"hb{c}"))
            pos = singles.tile([128, 512], F32, tag="pos", name="pos")
            dbtw = singles.tile([128, 512], F32, tag="dbtw", name="dbtw")
            wh = singles.tile([128, 2, 768], BF16, tag="wh", name="wh")
            wt = singles.tile([8, 6, 128], BF16, tag="wt", name="wt")
            wo = singles.tile([128, 2, 2], BF16, tag="wo", name="wo")
            wd2 = singles.tile([128, 128], F32, tag="wd2", name="wd2")
            bv = singles.tile([128, 8], F32, tag="bv", name="bv")
            bpk = singles.tile([2, 1], F32, tag="bpk", name="bpk")
            nc.vector.memset(dbtw, 0.0)

            # initial loads
            for c in range(CH):
                cs = slice(c * 512, (c + 1) * 512)
                nc.sync.dma_start(
                    out=h_c[c],
                    in_=h0_d[:, :, :].transpose([1, 0, 2])[:, :, cs])
                nc.sync.dma_start(
                    out=hb_c[c],
                    in_=h0b_d[:, :, :].transpose([1, 0, 2])[:, :, cs])
                nc.sync.dma_start(out=x_c[c], in_=x0_d[:, :][:, cs])
            nc.sync.dma_start(out=pos, in_=pos0_d[:, :])
            nc.sync.dma_start(out=wh, in_=wh_d[:, :, :].transpose([1, 0, 2]))
            nc.sync.dma_start(out=wt, in_=wt_d[:, :, :])
            nc.sync.dma_start(out=wo, in_=wo_d[:, :, :].transpose([1, 0, 2]))
            nc.sync.dma_start(out=wd2, in_=wd2_d[:, :])
            nc.sync.dma_start(out=bv, in_=bv_d[:, :])
            nc.sync.dma_start(out=bpk, in_=bpk_d[:, :])

            # pools
            pp1 = ctx.enter_context(tc.tile_pool(name="pp1", bufs=4, space="PSUM"))
            pp2 = ctx.enter_context(tc.tile_pool(name="pp2", bufs=2, space="PSUM"))
            pp3 = ctx.enter_context(tc.tile_pool(name="pp3", bufs=1, space="PSUM"))
            ppd = ctx.enter_context(tc.tile_pool(name="ppd", bufs=1, space="PSUM"))
            sb = ctx.enter_context(tc.tile_pool(name="sb", bufs=3))
            sbs = ctx.enter_context(tc.tile_pool(name="sbs", bufs=3))

            def step(t_idx):
                for c in range(CH):
                    hc = h_c[c]
                    hb = hb_c[c]
                    xc = x_c[c]
                    # --- P1: rz preactivations, 4 M-tiles ---
                    rzs = sb.tile([128, 4, 512], F32, tag="rzs", name="rzs")
                    for mt in range(4):
                        p1 = pp1.tile([128, 512], F32, tag="p1", name="p1")
                        ms_ = slice(mt * 128, (mt + 1) * 128)
                        nc.tensor.matmul(
                            p1, wh[:, 0, ms_],
                            hb[:, 0, :], start=True, stop=False)
                        nc.tensor.matmul(
                            p1, wh[:, 1, ms_],
                            hb[:, 1, :], start=False, stop=False)
                        nc.tensor.matmul(
                            p1, wt[0:7, mt, :],
                            xc[0:7, :],
                            start=False, stop=True)
                        nc.scalar.activation(
                            rzs[:, mt, :], p1, AF.Sigmoid,
                            bias=bv[:, mt:mt + 1])
                    # --- P2: i_n, P3: h_n ---
                    p2s, p3s = [], []
                    for i in range(2):
                        p2 = pp2.tile([128, 512], F32, tag="p2", name="p2")
                        nc.tensor.matmul(
                            p2, wt[0:7, 4 + i, :],
                            xc[0:7, :],
                            start=True, stop=True)
                        p2s.append(p2)
                    for i in range(2):
                        p3 = pp3.tile([128, 512], F32, tag="p3", name="p3")
                        ms_ = slice(512 + i * 128, 512 + (i + 1) * 128)
                        nc.tensor.matmul(
                            p3, wh[:, 0, ms_],
                            hb[:, 0, :], start=True, stop=False)
                        nc.tensor.matmul(
                            p3, wh[:, 1, ms_],
                            hb[:, 1, :], start=False, stop=True)
                        p3s.append(p3)
                    # --- npre = (P2 + b_ihn) + r*(P3 + b_hhn); n = tanh ---
                    npre = sb.tile([128, 2, 512], F32, tag="npre", name="npre")
                    for i in range(2):
                        t1 = sbs.tile([128, 512], F32, tag="t1", name="t1")
                        nc.vector.scalar_tensor_tensor(
                            t1, p3s[i], bv[:, 6 + i:7 + i], rzs[:, i, :],
                            op0=OP.add, op1=OP.mult)
                        nc.vector.scalar_tensor_tensor(
                            npre[:, i, :], p2s[i], bv[:, 4 + i:5 + i], t1,
                            op0=OP.add, op1=OP.add)
                    n_t = sb.tile([128, 2, 512], F32, tag="n", name="n")
                    for i in range(2):
                        nc.scalar.activation(
                            n_t[:, i, :], npre[:, i, :], AF.Tanh)
                    # --- h = (1-z)*n + z*h  (Pool only: TS + TT ops) ---
                    for kt in range(2):
                        eng = nc.gpsimd
                        zc = sbs.tile([128, 512], F32, tag=f"zc{kt}", name=f"zc{kt}")
                        eng.tensor_scalar(
                            zc, rzs[:, 2 + kt, :], -1.0, 1.0,
                            op0=OP.mult, op1=OP.add)
                        d_t = sbs.tile([128, 512], F32, tag=f"d{kt}", name=f"d{kt}")
                        eng.tensor_mul(d_t, zc, n_t[:, kt, :])
                        u_t = sbs.tile([128, 512], F32, tag=f"u{kt}", name=f"u{kt}")
                        eng.tensor_mul(u_t, rzs[:, 2 + kt, :], hc[:, kt, :])
                        eng.tensor_add(hc[:, kt, :], d_t, u_t)
                        nc.gpsimd.tensor_copy(hb[:, kt, :], hc[:, kt, :])
                    # --- delta = W_out @ h_new, spread eviction ---
                    pd = ppd.tile([2, 512], F32, tag="pdu", name="pdu")
                    nc.tensor.matmul(pd, wo[:, 0, :],
                                     hb[:, 0, :],
                                     start=True, stop=False)
                    nc.tensor.matmul(pd, wo[:, 1, :],
                                     hb[:, 1, :],
                                     start=False, stop=True)
                    nc.vector.tensor_scalar(
                        dbtw[32 * c:32 * c + 2, :], pd, bpk[0:2, :], None,
                        op0=OP.add)

                # ---- clip: s = min(MS/||delta||, 1), spread [128, 512] ----
                sqv = sbs.tile([128, 512], F32, tag="sqv", name="sqv")
                nc.gpsimd.tensor_mul(sqv, dbtw, dbtw)
                pu = ppd.tile([128, 512], F32, tag="pdu", name="pu")
                nc.tensor.matmul(pu, wd2, sqv, start=True, stop=True)
                s1i = sbs.tile([128, 512], I32, tag="s1i", name="s1i")
                nc.vector.tensor_scalar(
                    s1i, pu.bitcast(I32), 1, 0x3FFFFFFF,
                    op0=OP.logical_shift_right, op1=OP.bitwise_and)
                y0i = sbs.tile([128, 512], I32, tag="y0i", name="y0i")
                nc.vector.tensor_scalar(
                    y0i, s1i, MAGIC, -1, op0=OP.subtract, op1=OP.mult)
                y = y0i.bitcast(F32)
                for it in range(2):
                    m_t = sbs.tile([128, 512], F32, tag=f"m{it}", name=f"m{it}")
                    nc.vector.tensor_mul(m_t, y, y)
                    m2_t = sbs.tile([128, 512], F32, tag=f"m2{it}", name=f"m2{it}")
                    nc.vector.tensor_mul(m2_t, m_t, pu)
                    y2_t = sbs.tile([128, 512], F32, tag=f"y2{it}", name=f"y2{it}")
                    nc.vector.scalar_tensor_tensor(
                        y2_t, m2_t, 1.5, y, op0=OP.add, op1=OP.mult)
                    y = y2_t
                    if it == 0:
                        # x feedback tolerates 1-Newton precision (it is
                        # bf16-rounded anyway) -> unblock next step early
                        smin1 = sbs.tile([128, 512], F32, tag="smin1",
                                         name="smin1")
                        nc.vector.tensor_scalar(
                            smin1, y, 1.0, None, op0=OP.min)
                        for c in range(CH):
                            eng = nc.vector if c % 2 == 0 else nc.gpsimd
                            eng.tensor_mul(
                                x_c[c][0:2, :], smin1[32 * c:32 * c + 2, :],
                                dbtw[32 * c:32 * c + 2, :])
                # pos/output keep the 2-Newton value
                smin = sbs.tile([128, 512], F32, tag="smin", name="smin")
                nc.gpsimd.tensor_scalar(smin, y, 1.0, None, op0=OP.min)
                dct = sbs.tile([128, 512], F32, tag="dct", name="dct")
                nc.gpsimd.tensor_mul(dct, smin, dbtw)
                nc.gpsimd.tensor_add(pos, pos, dct)
                pos16 = sbs.tile([128, 512], F16, tag="pos16", name="pos16")
                nc.vector.tensor_copy(pos16, pos)
                for c in range(CH):
                    nc.sync.dma_start(
                        out=out_d[t_idx, 2 * c:2 * c + 2, :],
                        in_=pos16[32 * c:32 * c + 2, :])

            assert T % unroll == 0
            if repeats == 1:
                with tc.For_i(0, T, unroll) as iv:
                    for j in range(unroll):
                        step(iv + j)
            else:
                with tc.For_i(0, repeats, 1):
                    with tc.For_i(0, T, unroll) as iv:
                        for j in range(unroll):
                            step(iv + j)

    nc.finalize()
    return nc


# ---------------- host side ----------------

_module_cache: dict = {}


def _get_module(T: int, nloc: int, unroll: int, t_alloc: int | None = None,
                repeats: int = 1):
    key = (T, nloc, unroll, t_alloc, repeats)
    if key not in _module_cache:
        _module_cache[key] = build_module(T, nloc, unroll, t_alloc, repeats)
    return _module_cache[key]


def _host_prep(inputs, nloc):
    """Build per-core in_maps from full inputs."""
    N = inputs["init_h"].shape[0]
    n_sh = N // N_CORES
    CH = nloc // 512
    W_ih = np.asarray(inputs["W_ih"], np.float32)
    W_hh = np.asarray(inputs["W_hh"], np.float32)
    b_ih = np.asarray(inputs["b_ih"], np.float32)
    b_hh = np.asarray(inputs["b_hh"], np.float32)
    W_out = np.asarray(inputs["W_out"], np.float32)
    b_out = np.asarray(inputs["b_out"], np.float32)

    import ml_dtypes
    bf16 = ml_dtypes.bfloat16
    wh = np.ascontiguousarray(W_hh.T.reshape(2, 128, 768)).astype(bf16)
    wo = np.ascontiguousarray(W_out.T.reshape(2, 128, 2)).astype(bf16)

    # K=7 input tails: rows 0-1 = delta cols of W_ih, rows 2-6 = ctx cols
    wt = np.zeros((8, 6, 128), bf16)
    for mt in range(6):
        if mt < 4:
            rows = slice(mt * 128, (mt + 1) * 128)
        else:
            rows = slice(512 + (mt - 4) * 128, 512 + (mt - 3) * 128)
        wt[0:7, mt, :] = W_ih[rows, :].T.astype(bf16)

    # biases: cols 0-3 = (b_ih+b_hh) rz tiles, 4-5 = b_ih n, 6-7 = b_hh n
    bv = np.zeros((128, 8), np.float32)
    for mt in range(4):
        bv[:, mt] = (b_ih + b_hh)[mt * 128:(mt + 1) * 128]
    for i in range(2):
        bv[:, 4 + i] = b_ih[512 + i * 128:512 + (i + 1) * 128]
        bv[:, 6 + i] = b_hh[512 + i * 128:512 + (i + 1) * 128]

    wd2 = np.zeros((128, 128), np.float32)
    for c in range(CH):
        for i in range(2):
            for j in range(2):
                wd2[32 * c + i, 32 * c + j] = -0.5 / (MS * MS)

    bpk = np.asarray(b_out, np.float32).reshape(2, 1)

    init_h = np.asarray(inputs["init_h"], np.float32)
    ctx_in = np.asarray(inputs["ctx"], np.float32)
    x0 = np.asarray(inputs["x0"], np.float32)
    y0 = np.asarray(inputs["y0"], np.float32)

    in_maps = []
    for core in range(N_CORES):
        sl = slice(core * n_sh, (core + 1) * n_sh)
        h0 = np.ascontiguousarray(init_h[sl].T.reshape(2, 128, nloc))
        h0b = h0.astype(bf16)
        x0i = np.zeros((8, nloc), bf16)
        x0i[2:7] = ctx_in[sl].T.astype(bf16)
        pos0 = np.zeros((128, 512), np.float32)
        for c in range(CH):
            pos0[32 * c + 0] = x0[sl].reshape(CH, 512)[c]
            pos0[32 * c + 1] = y0[sl].reshape(CH, 512)[c]
        in_maps.append({
            "h0": h0, "h0b": h0b, "x0i": x0i, "pos0": pos0, "wh": wh,
            "wt": wt, "wo": wo, "wd2": wd2, "bv": bv, "bpk": bpk,
        })
    return in_maps


def _host_unpack(results, T, nloc):
    CH = nloc // 512
    outs = []
    for r in results:
        arr = np.asarray(r["out"][:T], np.float32)  # [T, 2CH, 512] rows 2c+coord
        a = arr.reshape(T, CH, 2, 512).transpose(1, 3, 0, 2)  # ch, s, T, 2
        outs.append(a.reshape(nloc, T, 2))
    return np.concatenate(outs, axis=0)


def kernel(**inputs) -> np.ndarray:
    T = int(inputs["T"])
    N = inputs["init_h"].shape[0]
    nloc = N // N_CORES
    unroll = 2
    nc = _get_module(T, nloc, unroll)
    in_maps = _host_prep(inputs, nloc)
    res = run_bass_kernel_spmd(nc, in_maps, core_ids=list(range(N_CORES)))
    return _host_unpack(res.results, T, nloc)


# revision 5
# speedup vs baseline: 309.8846x; 1.5150x over previous
"""Trainium2 Bass kernel for nn_Decoder (GRU decoder with clipped-delta
position integration).

Strategy
--------
Data-parallel over the batch N=16384: 8 cores x 2048 rows. Everything on-chip
per core runs in a *transposed* (feature-major) layout so the recurrent
matmul W_hh @ h streams h as the moving operand with weights stationary and
no per-step transposes are needed:

  h      [HID=256, 2048]  as SBUF [128, 2, 512] x4 chunks  (K-tile, batch)
  gates  [768, chunk=512] in PSUM, gate-major
  x_c    [8, 512] per chunk: rows 0-1 = prev delta (dx, dy), rows 2-6 = ctx.

Per step, per 512-column chunk:
  P1[mt<4] = W_hh[0:512] @ h + W_ih_aug @ x     (rz preact, PSUM; K=7 tail)
  P2[2]    = W_ih_aug @ x                       (i_n contribution)
  P3[2]    = W_hh[512:768] @ h                  (h_n contribution)
  r,z = sigmoid(P1 + b_rz)  -- biases ride the free per-partition ACT bias
  npre = (P2 + b_ihn) + r*(P3 + b_hhn)          -- biases via STT scalar APs
  n = tanh(npre); h = (1-z)*n + z*h on GPSIMD (Pool has no subtract/STT,
  so 1-z = (z*-1)+1 via tensor_scalar). h keeps an fp32 master copy plus a
  bf16 shadow (refreshed by one Pool copy per chunk) that feeds the PE.
  delta = W_out @ h_new  -> bias-add eviction into a spread [128,512] tile
                            (chunk c pair at partitions 32c, 32c+1).
Step-level clip: pair-sum matmul gives u' = -0.5*|d|^2/max_step^2 spread over
the same partitions; quake-seed + 2 Newton iterations on the DVE give
rsqrt(u) (no ACT table switch, all lanes busy); delta_clipped feeds pos
(+= on GPSIMD) and is written back into the x tiles by 32-aligned
DVE/Pool muls (no DMAs on the recurrence path).

The T loop is a hardware For_i loop (unrolled by `unroll` steps per
iteration) so the program size is O(1) in T; outputs are written per step
as fp16 into a [t_alloc, 8, 512] DRAM tensor (4 small DMAs/step).
`repeats` wraps the T loop in an outer hardware loop for timing runs
(state just keeps evolving; used only by test.py to amplify the per-step
signal above the remote-dispatch noise floor).

Matmul operands are bf16 (1 col/cycle on the PE, FWL weight loads; fp32
runs at 1/4 rate and float32r trips walrus sync-wait limits); PSUM
accumulation is fp32 and the norm matmul stays fp32 for precision.
"""

import sys

for _p in ("/opt/trn_rl_repo",):
    if _p not in sys.path:
        sys.path.insert(0, _p)

import numpy as np

import concourse.bass as bass
import concourse.tile as tile
from concourse.bacc import Bacc
from concourse import mybir
from concourse.bass_utils import run_bass_kernel_spmd

F32 = mybir.dt.float32
F16 = mybir.dt.float16
BF16 = mybir.dt.bfloat16
I32 = mybir.dt.int32
AF = mybir.ActivationFunctionType
OP = mybir.AluOpType

HID = 256
CTX_DIM = 5
V_MAX = 10.1415
DT = 0.093
MS = V_MAX * DT  # max_step
N_CORES = 8
MAGIC = 0x5F3759DF - 0x400000  # quake magic adjusted for input u' = -0.5*u

ABLATE: set = set()  # sim-only knobs: {'clip','hupd','act','npre','evict'}


def build_module(T: int, nloc: int, unroll: int = 2, t_alloc: int | None = None,
                 repeats: int = 1):
    """Trace the Bass/Tile module for one core (nloc batch columns)."""
    CH = nloc // 512  # column chunks of 512
    assert nloc % 512 == 0
    if t_alloc is None:
        t_alloc = T

    nc = Bacc()

    # ---- DRAM I/O ----
    h0_d = nc.dram_tensor("h0", [2, 128, nloc], F32, kind="ExternalInput")
    h0b_d = nc.dram_tensor("h0b", [2, 128, nloc], BF16, kind="ExternalInput")
    x0_d = nc.dram_tensor("x0i", [8, nloc], BF16, kind="ExternalInput")
    pos0_d = nc.dram_tensor("pos0", [128, 512], F32, kind="ExternalInput")
    wh_d = nc.dram_tensor("wh", [2, 128, 768], BF16, kind="ExternalInput")
    wt_d = nc.dram_tensor("wt", [8, 6, 128], BF16, kind="ExternalInput")
    wo_d = nc.dram_tensor("wo", [2, 128, 2], BF16, kind="ExternalInput")
    wd2_d = nc.dram_tensor("wd2", [128, 128], F32, kind="ExternalInput")
    bv_d = nc.dram_tensor("bv", [128, 8], F32, kind="ExternalInput")
    bpk_d = nc.dram_tensor("bpk", [2, 1], F32, kind="ExternalInput")
    out_d = nc.dram_tensor("out", [t_alloc, 2 * CH, 512], F16,
                           kind="ExternalOutput")

    with tile.TileContext(nc) as tc:
        import contextlib

        ctx = contextlib.ExitStack()
        with ctx:
            singles = ctx.enter_context(tc.tile_pool(name="singles", bufs=1))
            h_c = []
            x_c = []
            hb_c = []
            for c in range(CH):
                h_c.append(singles.tile([128, 2, 512], F32, tag=f"h{c}", name=f"h{c}"))
                x_c.append(singles.tile([8, 512], BF16, tag=f"x{c}", name=f"x{c}"))
                hb_c.append(singles.tile([128, 2, 512], BF16, tag=f"hb{c}", name=f"hb{c}"))
            pos = singles.tile([128, 512], F32, tag="pos", name="pos")
            dbtw = singles.tile([128, 512], F32, tag="dbtw", name="dbtw")
            wh = singles.tile([128, 2, 768], BF16, tag="wh", name="wh")
            wt = singles.tile([8, 6, 128], BF16, tag="wt", name="wt")
            wo = singles.tile([128, 2, 2], BF16, tag="wo", name="wo")
            wd2 = singles.tile([128, 128], F32, tag="wd2", name="wd2")
            bv = singles.tile([128, 8], F32, tag="bv", name="bv")
            bpk = singles.tile([2, 1], F32, tag="bpk", name="bpk")
            nc.vector.memset(dbtw, 0.0)

            # initial loads
            for c in range(CH):
                cs = slice(c * 512, (c + 1) * 512)
                nc.sync.dma_start(
                    out=h_c[c],
                    in_=h0_d[:, :, :].transpose([1, 0, 2])[:, :, cs])
                nc.sync.dma_start(
                    out=hb_c[c],
                    in_=h0b_d[:, :, :].transpose([1, 0, 2])[:, :, cs])
                nc.sync.dma_start(out=x_c[c], in_=x0_d[:, :][:, cs])
            nc.sync.dma_start(out=pos, in_=pos0_d[:, :])
            nc.sync.dma_start(out=wh, in_=wh_d[:, :, :].transpose([1, 0, 2]))
            nc.sync.dma_start(out=wt, in_=wt_d[:, :, :])
            nc.sync.dma_start(out=wo, in_=wo_d[:, :, :].transpose([1, 0, 2]))
            nc.sync.dma_start(out=wd2, in_=wd2_d[:, :])
            nc.sync.dma_start(out=bv, in_=bv_d[:, :])
            nc.sync.dma_start(out=bpk, in_=bpk_d[:, :])

            # pools
            pp1 = ctx.enter_context(tc.tile_pool(name="pp1", bufs=4, space="PSUM"))
            pp2 = ctx.enter_context(tc.tile_pool(name="pp2", bufs=2, space="PSUM"))
            pp3 = ctx.enter_context(tc.tile_pool(name="pp3", bufs=1, space="PSUM"))
            ppd = ctx.enter_context(tc.tile_pool(name="ppd", bufs=1, space="PSUM"))
            sb = ctx.enter_context(tc.tile_pool(name="sb", bufs=3))
            sbs = ctx.enter_context(tc.tile_pool(name="sbs", bufs=3))

            def step(t_idx):
                for c in range(CH):
                    hc = h_c[c]
                    hb = hb_c[c]
                    xc = x_c[c]
                    # --- P1: rz preactivations, 4 M-tiles ---
                    rzs = sb.tile([128, 4, 512], F32, tag="rzs", name="rzs")
                    for mt in range(4):
                        p1 = pp1.tile([128, 512], F32, tag="p1", name="p1")
                        ms_ = slice(mt * 128, (mt + 1) * 128)
                        nc.tensor.matmul(
                            p1, wh[:, 0, ms_],
                            hb[:, 0, :], start=True, stop=False)
                        nc.tensor.matmul(
                            p1, wh[:, 1, ms_],
                            hb[:, 1, :], start=False, stop=False)
                        nc.tensor.matmul(
                            p1, wt[0:7, mt, :],
                            xc[0:7, :],
                            start=False, stop=True)
                        if 'act' not in ABLATE:
                            nc.scalar.activation(
                                rzs[:, mt, :], p1, AF.Sigmoid,
                                bias=bv[:, mt:mt + 1])
                    # --- P2: i_n, P3: h_n ---
                    p2s, p3s = [], []
                    for i in range(2):
                        p2 = pp2.tile([128, 512], F32, tag="p2", name="p2")
                        nc.tensor.matmul(
                            p2, wt[0:7, 4 + i, :],
                            xc[0:7, :],
                            start=True, stop=True)
                        p2s.append(p2)
                    for i in range(2):
                        p3 = pp3.tile([128, 512], F32, tag="p3", name="p3")
                        ms_ = slice(512 + i * 128, 512 + (i + 1) * 128)
                        nc.tensor.matmul(
                            p3, wh[:, 0, ms_],
                            hb[:, 0, :], start=True, stop=False)
                        nc.tensor.matmul(
                            p3, wh[:, 1, ms_],
                            hb[:, 1, :], start=False, stop=True)
                        p3s.append(p3)
                    # --- npre = (P2 + b_ihn) + r*(P3 + b_hhn); n = tanh ---
                    npre = sb.tile([128, 2, 512], F32, tag="npre", name="npre")
                    if 'npre' not in ABLATE:
                        for i in range(2):
                            t1 = sbs.tile([128, 512], F32, tag="t1", name="t1")
                            nc.vector.scalar_tensor_tensor(
                                t1, p3s[i], bv[:, 6 + i:7 + i], rzs[:, i, :],
                                op0=OP.add, op1=OP.mult)
                            nc.vector.scalar_tensor_tensor(
                                npre[:, i, :], p2s[i], bv[:, 4 + i:5 + i], t1,
                                op0=OP.add, op1=OP.add)
                    n_t = sb.tile([128, 2, 512], F32, tag="n", name="n")
                    if 'act' not in ABLATE:
                        for i in range(2):
                            nc.scalar.activation(
                                n_t[:, i, :], npre[:, i, :], AF.Tanh)
                    # --- h = (1-z)*n + z*h  (Pool only: TS + TT ops) ---
                    if 'hupd' not in ABLATE:
                        for kt in range(2):
                            eng = nc.gpsimd
                            zc = sbs.tile([128, 512], F32, tag=f"zc{kt}", name=f"zc{kt}")
                            eng.tensor_scalar(
                                zc, rzs[:, 2 + kt, :], -1.0, 1.0,
                                op0=OP.mult, op1=OP.add)
                            d_t = sbs.tile([128, 512], F32, tag=f"d{kt}", name=f"d{kt}")
                            eng.tensor_mul(d_t, zc, n_t[:, kt, :])
                            u_t = sbs.tile([128, 512], F32, tag=f"u{kt}", name=f"u{kt}")
                            eng.tensor_mul(u_t, rzs[:, 2 + kt, :], hc[:, kt, :])
                            eng.tensor_add(hc[:, kt, :], d_t, u_t)
                            nc.gpsimd.tensor_copy(hb[:, kt, :], hc[:, kt, :])
                    # --- delta = W_out @ h_new, spread eviction ---
                    pd = ppd.tile([2, 512], F32, tag="pdu", name="pdu")
                    nc.tensor.matmul(pd, wo[:, 0, :],
                                     hb[:, 0, :],
                                     start=True, stop=False)
                    nc.tensor.matmul(pd, wo[:, 1, :],
                                     hb[:, 1, :],
                                     start=False, stop=True)
                    nc.vector.tensor_scalar(
                        dbtw[32 * c:32 * c + 2, :], pd, bpk[0:2, :], None,
                        op0=OP.add)

                # ---- clip: s = min(MS/||delta||, 1), spread [128, 512] ----
                if 'clip' in ABLATE:
                    nc.gpsimd.tensor_add(pos, pos, dbtw)
                    pos16 = sbs.tile([128, 512], F16, tag="pos16", name="pos16")
                    nc.vector.tensor_copy(pos16, pos)
                    for c in range(CH):
                        nc.sync.dma_start(
                            out=out_d[t_idx, 2 * c:2 * c + 2, :],
                            in_=pos16[32 * c:32 * c + 2, :])
                    return
                sqv = sbs.tile([128, 512], F32, tag="sqv", name="sqv")
                nc.gpsimd.tensor_mul(sqv, dbtw, dbtw)
                pu = ppd.tile([128, 512], F32, tag="pdu", name="pu")
                nc.tensor.matmul(pu, wd2, sqv, start=True, stop=True)
                s1i = sbs.tile([128, 512], I32, tag="s1i", name="s1i")
                nc.vector.tensor_scalar(
                    s1i, pu.bitcast(I32), 1, 0x3FFFFFFF,
                    op0=OP.logical_shift_right, op1=OP.bitwise_and)
                y0i = sbs.tile([128, 512], I32, tag="y0i", name="y0i")
                nc.vector.tensor_scalar(
                    y0i, s1i, MAGIC, -1, op0=OP.subtract, op1=OP.mult)
                y = y0i.bitcast(F32)
                for it in range(2):
                    m_t = sbs.tile([128, 512], F32, tag=f"m{it}", name=f"m{it}")
                    nc.vector.tensor_mul(m_t, y, y)
                    m2_t = sbs.tile([128, 512], F32, tag=f"m2{it}", name=f"m2{it}")
                    nc.vector.tensor_mul(m2_t, m_t, pu)
                    y2_t = sbs.tile([128, 512], F32, tag=f"y2{it}", name=f"y2{it}")
                    nc.vector.scalar_tensor_tensor(
                        y2_t, m2_t, 1.5, y, op0=OP.add, op1=OP.mult)
                    y = y2_t
                    if it == 0:
                        # x feedback tolerates 1-Newton precision (it is
                        # bf16-rounded anyway) -> unblock next step early
                        smin1 = sbs.tile([128, 512], F32, tag="smin1",
                                         name="smin1")
                        nc.vector.tensor_scalar(
                            smin1, y, 1.0, None, op0=OP.min)
                        for c in range(CH):
                            eng = nc.vector if c % 2 == 0 else nc.gpsimd
                            eng.tensor_mul(
                                x_c[c][0:2, :], smin1[32 * c:32 * c + 2, :],
                                dbtw[32 * c:32 * c + 2, :])
                # pos/output keep the 2-Newton value
                smin = sbs.tile([128, 512], F32, tag="smin", name="smin")
                nc.gpsimd.tensor_scalar(smin, y, 1.0, None, op0=OP.min)
                dct = sbs.tile([128, 512], F32, tag="dct", name="dct")
                nc.gpsimd.tensor_mul(dct, smin, dbtw)
                nc.gpsimd.tensor_add(pos, pos, dct)
                pos16 = sbs.tile([128, 512], F16, tag="pos16", name="pos16")
                nc.vector.tensor_copy(pos16, pos)
                for c in range(CH):
                    nc.sync.dma_start(
                        out=out_d[t_idx, 2 * c:2 * c + 2, :],
                        in_=pos16[32 * c:32 * c + 2, :])

            if unroll == 0:  # static python loop (TimelineSim / debug)
                for t in range(T):
                    step(t)
            else:
                assert T % unroll == 0
                if repeats == 1:
                    with tc.For_i(0, T, unroll) as iv:
                        for j in range(unroll):
                            step(iv + j)
                else:
                    with tc.For_i(0, repeats, 1):
                        with tc.For_i(0, T, unroll) as iv:
                            for j in range(unroll):
                                step(iv + j)

    nc.finalize()
    return nc


# ---------------- host side ----------------

_module_cache: dict = {}


def _get_module(T: int, nloc: int, unroll: int, t_alloc: int | None = None,
                repeats: int = 1):
    key = (T, nloc, unroll, t_alloc, repeats)
    if key not in _module_cache:
        _module_cache[key] = build_module(T, nloc, unroll, t_alloc, repeats)
    return _module_cache[key]


def _host_prep(inputs, nloc):
    """Build per-core in_maps from full inputs."""
    N = inputs["init_h"].shape[0]
    n_sh = N // N_CORES
    CH = nloc // 512
    W_ih = np.asarray(inputs["W_ih"], np.float32)
    W_hh = np.asarray(inputs["W_hh"], np.float32)
    b_ih = np.asarray(inputs["b_ih"], np.float32)
    b_hh = np.asarray(inputs["b_hh"], np.float32)
    W_out = np.asarray(inputs["W_out"], np.float32)
    b_out = np.asarray(inputs["b_out"], np.float32)

    import ml_dtypes
    bf16 = ml_dtypes.bfloat16
    wh = np.ascontiguousarray(W_hh.T.reshape(2, 128, 768)).astype(bf16)
    wo = np.ascontiguousarray(W_out.T.reshape(2, 128, 2)).astype(bf16)

    # K=7 input tails: rows 0-1 = delta cols of W_ih, rows 2-6 = ctx cols
    wt = np.zeros((8, 6, 128), bf16)
    for mt in range(6):
        if mt < 4:
            rows = slice(mt * 128, (mt + 1) * 128)
        else:
            rows = slice(512 + (mt - 4) * 128, 512 + (mt - 3) * 128)
        wt[0:7, mt, :] = W_ih[rows, :].T.astype(bf16)

    # biases: cols 0-3 = (b_ih+b_hh) rz tiles, 4-5 = b_ih n, 6-7 = b_hh n
    bv = np.zeros((128, 8), np.float32)
    for mt in range(4):
        bv[:, mt] = (b_ih + b_hh)[mt * 128:(mt + 1) * 128]
    for i in range(2):
        bv[:, 4 + i] = b_ih[512 + i * 128:512 + (i + 1) * 128]
        bv[:, 6 + i] = b_hh[512 + i * 128:512 + (i + 1) * 128]

    wd2 = np.zeros((128, 128), np.float32)
    for c in range(CH):
        for i in range(2):
            for j in range(2):
                wd2[32 * c + i, 32 * c + j] = -0.5 / (MS * MS)

    bpk = np.asarray(b_out, np.float32).reshape(2, 1)

    init_h = np.asarray(inputs["init_h"], np.float32)
    ctx_in = np.asarray(inputs["ctx"], np.float32)
    x0 = np.asarray(inputs["x0"], np.float32)
    y0 = np.asarray(inputs["y0"], np.float32)

    in_maps = []
    for core in range(N_CORES):
        sl = slice(core * n_sh, (core + 1) * n_sh)
        h0 = np.ascontiguousarray(init_h[sl].T.reshape(2, 128, nloc))
        h0b = h0.astype(bf16)
        x0i = np.zeros((8, nloc), bf16)
        x0i[2:7] = ctx_in[sl].T.astype(bf16)
        pos0 = np.zeros((128, 512), np.float32)
        for c in range(CH):
            pos0[32 * c + 0] = x0[sl].reshape(CH, 512)[c]
            pos0[32 * c + 1] = y0[sl].reshape(CH, 512)[c]
        in_maps.append({
            "h0": h0, "h0b": h0b, "x0i": x0i, "pos0": pos0, "wh": wh,
            "wt": wt, "wo": wo, "wd2": wd2, "bv": bv, "bpk": bpk,
        })
    return in_maps


def _host_unpack(results, T, nloc):
    CH = nloc // 512
    outs = []
    for r in results:
        arr = np.asarray(r["out"][:T], np.float32)  # [T, 2CH, 512] rows 2c+coord
        a = arr.reshape(T, CH, 2, 512).transpose(1, 3, 0, 2)  # ch, s, T, 2
        outs.append(a.reshape(nloc, T, 2))
    return np.concatenate(outs, axis=0)


def kernel(**inputs) -> np.ndarray:
    T = int(inputs["T"])
    N = inputs["init_h"].shape[0]
    nloc = N // N_CORES
    unroll = 2
    nc = _get_module(T, nloc, unroll)
    in_maps = _host_prep(inputs, nloc)
    res = run_bass_kernel_spmd(nc, in_maps, core_ids=list(range(N_CORES)))
    return _host_unpack(res.results, T, nloc)


# revision 6
# speedup vs baseline: 385.7729x; 1.2449x over previous
"""Trainium2 Bass kernel for nn_Decoder — v2 (bf16 gates/state, short chains).

Same transposed data-parallel layout as v1 (8 cores x 2048 batch cols,
feature-major on chip), with these changes:

- h is stored ONLY in bf16 (no fp32 master + shadow copy): the GRU map is
  contractive and the output tolerance absorbs the extra ~2e-3.
- All gate tensors (r, z, n) and the h-update intermediates are bf16 so
  DVE tensor_tensor ops run in 2x mode and ACT evictions can pack.
- h update is 3 ops via h' = n + z*(h-n): STT(h-n), TT(z*...), TT(n+...),
  pairs split between DVE and Pool.
- clip uses a single Newton iteration (seed err 3.4% -> 0.17%), with the
  bf16-rounding of the norm weight compensated exactly by a constant
  factor folded into the final min(k*y, 1) tensor_scalar.
- matmul issue order keeps PE streaming: per chunk all W_hh passes, then
  the K=7 input tails; all W_out matmuls after every chunk's gates.
- T loop is a hardware For_i (program size O(1) in T); out is fp16
  [t_alloc, 8, 512]; `repeats` wraps the loop for timing runs.
"""

import sys

for _p in ("/opt/trn_rl_repo",):
    if _p not in sys.path:
        sys.path.insert(0, _p)

import numpy as np

import concourse.bass as bass
import concourse.tile as tile
from concourse.bacc import Bacc
from concourse import mybir
from concourse.bass_utils import run_bass_kernel_spmd

F32 = mybir.dt.float32
F16 = mybir.dt.float16
BF16 = mybir.dt.bfloat16
I32 = mybir.dt.int32
AF = mybir.ActivationFunctionType
OP = mybir.AluOpType

HID = 256
CTX_DIM = 5
V_MAX = 10.1415
DT = 0.093
MS = V_MAX * DT  # max_step
N_CORES = 8
MAGIC = 0x5F3759DF - 0x400000  # quake magic adjusted for input u' = -0.5*u

ABLATE: set = set()  # sim-only knobs

# bf16 rounding of the norm-matmul constant, compensated in the final min:
# u_psum = c_bf * |d|^2 where c_bf = bf16(-0.5/MS^2); the Newton result is
# y ~= rsqrt(-0.5 * u_psum) = rsqrt(0.25*|d|^2/MS^2 * (c_bf/c_exact)) ...
# we need s = MS/|d| = rsqrt(|d|^2/MS^2), and the magic-seed pipeline
# computes rsqrt for input u' = -0.5*u with u = |d|^2/MS^2.  With the bf16
# weight the effective u' is scaled by rho = c_bf/c_exact, so y = true/sqrt(rho)
# and s = y * sqrt(rho).
import ml_dtypes as _mld
_C_EXACT = -0.5 / (MS * MS)
_C_BF = float(np.float32(_mld.bfloat16(_C_EXACT)))
_KCOMP = float(np.sqrt(_C_BF / _C_EXACT))


def build_module(T: int, nloc: int, unroll: int = 2, t_alloc: int | None = None,
                 repeats: int = 1):
    """Trace the Bass/Tile module for one core (nloc batch columns)."""
    CH = nloc // 512  # column chunks of 512
    assert nloc % 512 == 0
    if t_alloc is None:
        t_alloc = T

    nc = Bacc()

    # ---- DRAM I/O ----
    h0b_d = nc.dram_tensor("h0b", [2, 128, nloc], BF16, kind="ExternalInput")
    x0_d = nc.dram_tensor("x0i", [8, nloc // 512, 512], BF16, kind="ExternalInput")
    pos0_d = nc.dram_tensor("pos0", [8, 512], F32, kind="ExternalInput")
    wh_d = nc.dram_tensor("wh", [2, 128, 768], BF16, kind="ExternalInput")
    wt_d = nc.dram_tensor("wt", [8, 6, 128], BF16, kind="ExternalInput")
    wo_d = nc.dram_tensor("wo", [128, 4, 2, 8], BF16, kind="ExternalInput")
    wd2_d = nc.dram_tensor("wd2", [8, 8], BF16, kind="ExternalInput")
    bv_d = nc.dram_tensor("bv", [128, 8], F32, kind="ExternalInput")
    bpk_d = nc.dram_tensor("bpk", [8, 1], F32, kind="ExternalInput")
    out_d = nc.dram_tensor("out", [t_alloc, 2 * CH, 512], F16,
                           kind="ExternalOutput")

    with tile.TileContext(nc) as tc:
        import contextlib

        ctx = contextlib.ExitStack()
        with ctx:
            singles = ctx.enter_context(tc.tile_pool(name="singles", bufs=1))
            hb_c = []
            for c in range(CH):
                hb_c.append(singles.tile([128, 2, 512], BF16, tag=f"hb{c}", name=f"hb{c}"))
            x_all = singles.tile([8, CH, 512], BF16, tag="xall", name="xall")
            pos = singles.tile([8, 512], F32, tag="pos", name="pos")
            dbtw = singles.tile([8, 512], F32, tag="dbtw", name="dbtw")
            wh = singles.tile([128, 2, 768], BF16, tag="wh", name="wh")
            wt = singles.tile([8, 6, 128], BF16, tag="wt", name="wt")
            wo = singles.tile([128, 4, 2, 8], BF16, tag="wo", name="wo")
            wd2 = singles.tile([8, 8], BF16, tag="wd2", name="wd2")
            bv = singles.tile([128, 8], F32, tag="bv", name="bv")
            bpk = singles.tile([8, 1], F32, tag="bpk", name="bpk")
            nc.vector.memset(dbtw, 0.0)

            # initial loads
            for c in range(CH):
                cs = slice(c * 512, (c + 1) * 512)
                nc.sync.dma_start(
                    out=hb_c[c],
                    in_=h0b_d[:, :, :].transpose([1, 0, 2])[:, :, cs])
            nc.sync.dma_start(out=x_all, in_=x0_d[:, :, :])
            nc.sync.dma_start(out=pos, in_=pos0_d[:, :])
            nc.sync.dma_start(out=wh, in_=wh_d[:, :, :].transpose([1, 0, 2]))
            nc.sync.dma_start(out=wt, in_=wt_d[:, :, :])
            nc.sync.dma_start(out=wo, in_=wo_d[:, :, :, :])
            nc.sync.dma_start(out=wd2, in_=wd2_d[:, :])
            nc.sync.dma_start(out=bv, in_=bv_d[:, :])
            nc.sync.dma_start(out=bpk, in_=bpk_d[:, :])

            # pools (PSUM: 4 + 2 + 1 + 1 = 8 banks)
            pp1 = ctx.enter_context(tc.tile_pool(name="pp1", bufs=4, space="PSUM"))
            pp2 = ctx.enter_context(tc.tile_pool(name="pp2", bufs=2, space="PSUM"))
            pp3 = ctx.enter_context(tc.tile_pool(name="pp3", bufs=1, space="PSUM"))
            ppd = ctx.enter_context(tc.tile_pool(name="ppd", bufs=1, space="PSUM"))
            sb = ctx.enter_context(tc.tile_pool(name="sb", bufs=3))
            sbs = ctx.enter_context(tc.tile_pool(name="sbs", bufs=3))

            def step(t_idx):
                rz_c, n_c, p1g, p2g, p3g = [], [], {}, {}, {}
                # ---- Phase A: gate matmuls, whh first, tails late ----
                for c in range(CH):
                    hb = hb_c[c]
                    for mt in range(4):
                        p1 = pp1.tile([128, 512], F32, tag="p1", name="p1")
                        ms_ = slice(mt * 128, (mt + 1) * 128)
                        nc.tensor.matmul(p1, wh[:, 0, ms_], hb[:, 0, :],
                                         start=True, stop=False)
                        nc.tensor.matmul(p1, wh[:, 1, ms_], hb[:, 1, :],
                                         start=False, stop=False)
                        p1g[(c, mt)] = p1
                    for i in range(2):
                        p3 = pp3.tile([128, 512], F32, tag="p3", name="p3")
                        ms_ = slice(512 + i * 128, 512 + (i + 1) * 128)
                        nc.tensor.matmul(p3, wh[:, 0, ms_], hb[:, 0, :],
                                         start=True, stop=False)
                        nc.tensor.matmul(p3, wh[:, 1, ms_], hb[:, 1, :],
                                         start=False, stop=True)
                        p3g[(c, i)] = p3
                    # K=7 input tails (need x from previous step's clip)
                    for mt in range(4):
                        nc.tensor.matmul(p1g[(c, mt)], wt[0:7, mt, :],
                                         x_all[0:7, c, :], start=False, stop=True)
                    for i in range(2):
                        p2 = pp2.tile([128, 512], F32, tag="p2", name="p2")
                        nc.tensor.matmul(p2, wt[0:7, 4 + i, :], x_all[0:7, c, :],
                                         start=True, stop=True)
                        p2g[(c, i)] = p2

                    # ---- gate evictions + n preact (bf16 outputs) ----
                    rzs = sb.tile([128, 4, 512], BF16, tag="rzs", name="rzs")
                    for mt in range(4):
                        nc.scalar.activation(rzs[:, mt, :], p1g[(c, mt)],
                                             AF.Sigmoid, bias=bv[:, mt:mt + 1])
                    n_t = sb.tile([128, 2, 512], BF16, tag="n", name="n")
                    for i in range(2):
                        t1 = sbs.tile([128, 512], BF16, tag="t1", name="t1")
                        nc.vector.scalar_tensor_tensor(
                            t1, p3g[(c, i)], bv[:, 6 + i:7 + i], rzs[:, i, :],
                            op0=OP.add, op1=OP.mult)
                        npre = sbs.tile([128, 512], F32, tag="npre", name="npre")
                        nc.vector.scalar_tensor_tensor(
                            npre, p2g[(c, i)], bv[:, 4 + i:5 + i], t1,
                            op0=OP.add, op1=OP.add)
                        nc.scalar.activation(n_t[:, i, :], npre, AF.Tanh)
                    rz_c.append(rzs)
                    n_c.append(n_t)

                    # ---- h' = n + z*(h - n), bf16, 3 ops per k-tile ----
                    for kt in range(2 if 'hupd' not in ABLATE else 0):
                        d_eng, p_eng = ((nc.vector, nc.gpsimd)
                                        if (c + kt) % 2 == 0
                                        else (nc.gpsimd, nc.vector))
                        hmn = sbs.tile([128, 512], BF16, tag=f"hmn{kt}",
                                       name=f"hmn{kt}")
                        nc.vector.scalar_tensor_tensor(
                            hmn, n_t[:, kt, :], -1.0, hb[:, kt, :],
                            op0=OP.mult, op1=OP.add)
                        znm = sbs.tile([128, 512], BF16, tag=f"znm{kt}",
                                       name=f"znm{kt}")
                        d_eng.tensor_mul(znm, rzs[:, 2 + kt, :], hmn)
                        p_eng.tensor_add(hb[:, kt, :], n_t[:, kt, :], znm)

                # ---- W_out after all gates: deltas into one [8,512] bank ----
                pd = ppd.tile([8, 512], F32, tag="pdu", name="pdu")
                for c in range(CH):
                    nc.tensor.matmul(pd, wo[:, c, 0, :], hb_c[c][:, 0, :],
                                     start=(c == 0), stop=False)
                    nc.tensor.matmul(pd, wo[:, c, 1, :], hb_c[c][:, 1, :],
                                     start=False, stop=(c == CH - 1))
                nc.vector.tensor_scalar(dbtw, pd, bpk, None, op0=OP.add)

                # ---- clip: s = min(k*rsqrt_1newton(u'), 1) ----
                if 'clip' in ABLATE:
                    nc.gpsimd.tensor_add(pos, pos, dbtw)
                    pos16 = sbs.tile([8, 512], F16, tag="pos16", name="pos16")
                    nc.gpsimd.tensor_copy(pos16, pos)
                    nc.sync.dma_start(out=out_d[t_idx, :, :], in_=pos16)
                    return
                sqv = sbs.tile([8, 512], BF16, tag="sqv", name="sqv")
                nc.gpsimd.tensor_mul(sqv, dbtw, dbtw)
                pu = ppd.tile([8, 512], F32, tag="pdu", name="pu")
                nc.tensor.matmul(pu, wd2, sqv, start=True, stop=True)
                s1i = sbs.tile([8, 512], I32, tag="s1i", name="s1i")
                nc.vector.tensor_scalar(
                    s1i, pu.bitcast(I32), 1, 0x3FFFFFFF,
                    op0=OP.logical_shift_right, op1=OP.bitwise_and)
                y0i = sbs.tile([8, 512], I32, tag="y0i", name="y0i")
                nc.vector.tensor_scalar(
                    y0i, s1i, MAGIC, -1, op0=OP.subtract, op1=OP.mult)
                y0 = y0i.bitcast(F32)
                m_t = sbs.tile([8, 512], F32, tag="m", name="m")
                nc.vector.tensor_mul(m_t, y0, y0)
                m2_t = sbs.tile([8, 512], F32, tag="m2", name="m2")
                nc.vector.tensor_mul(m2_t, m_t, pu)
                y2_t = sbs.tile([8, 512], F32, tag="y2", name="y2")
                nc.vector.scalar_tensor_tensor(
                    y2_t, m2_t, 1.5, y0, op0=OP.add, op1=OP.mult)
                smin = sbs.tile([8, 512], F32, tag="smin", name="smin")
                nc.vector.tensor_scalar(
                    smin, y2_t, _KCOMP, 1.0, op0=OP.mult, op1=OP.min)
                # x feedback (bf16) for next step's tails: one op + DMAs
                xprod = sbs.tile([8, 512], BF16, tag="xprod", name="xprod")
                nc.vector.tensor_mul(xprod, smin, dbtw)
                for c in range(CH):
                    nc.sync.dma_start(out=x_all[0:2, c, :],
                                      in_=xprod[2 * c:2 * c + 2, :])
                # pos path (off the recurrence): fp32 accumulate, fp16 out
                dct = sbs.tile([8, 512], F32, tag="dct", name="dct")
                nc.gpsimd.tensor_mul(dct, smin, dbtw)
                nc.gpsimd.tensor_add(pos, pos, dct)
                pos16 = sbs.tile([8, 512], F16, tag="pos16", name="pos16")
                nc.gpsimd.tensor_copy(pos16, pos)
                nc.sync.dma_start(out=out_d[t_idx, :, :], in_=pos16)

            if unroll == 0:  # static python loop (TimelineSim / debug)
                for t in range(T):
                    step(t)
            else:
                assert T % unroll == 0
                if repeats == 1:
                    with tc.For_i(0, T, unroll) as iv:
                        for j in range(unroll):
                            step(iv + j)
                else:
                    with tc.For_i(0, repeats, 1):
                        with tc.For_i(0, T, unroll) as iv:
                            for j in range(unroll):
                                step(iv + j)

    nc.finalize()
    return nc


# ---------------- host side ----------------

_module_cache: dict = {}


def _get_module(T: int, nloc: int, unroll: int, t_alloc: int | None = None,
                repeats: int = 1):
    key = (T, nloc, unroll, t_alloc, repeats)
    if key not in _module_cache:
        _module_cache[key] = build_module(T, nloc, unroll, t_alloc, repeats)
    return _module_cache[key]


def _host_prep(inputs, nloc):
    """Build per-core in_maps from full inputs."""
    N = inputs["init_h"].shape[0]
    n_sh = N // N_CORES
    CH = nloc // 512
    W_ih = np.asarray(inputs["W_ih"], np.float32)
    W_hh = np.asarray(inputs["W_hh"], np.float32)
    b_ih = np.asarray(inputs["b_ih"], np.float32)
    b_hh = np.asarray(inputs["b_hh"], np.float32)
    W_out = np.asarray(inputs["W_out"], np.float32)
    b_out = np.asarray(inputs["b_out"], np.float32)

    import ml_dtypes
    bf16 = ml_dtypes.bfloat16
    wh = np.ascontiguousarray(W_hh.T.reshape(2, 128, 768)).astype(bf16)
    woT = W_out.T.reshape(2, 128, 2)  # [kt, 128, i]
    wo = np.zeros((128, 4, 2, 8), np.float32)
    for c in range(4):
        for kt in range(2):
            for i in range(2):
                wo[:, c, kt, 2 * c + i] = woT[kt, :, i]
    wo = wo.astype(bf16)

    # K=7 input tails: rows 0-1 = delta cols of W_ih, rows 2-6 = ctx cols
    wt = np.zeros((8, 6, 128), bf16)
    for mt in range(6):
        if mt < 4:
            rows = slice(mt * 128, (mt + 1) * 128)
        else:
            rows = slice(512 + (mt - 4) * 128, 512 + (mt - 3) * 128)
        wt[0:7, mt, :] = W_ih[rows, :].T.astype(bf16)

    # biases: cols 0-3 = (b_ih+b_hh) rz tiles, 4-5 = b_ih n, 6-7 = b_hh n
    bv = np.zeros((128, 8), np.float32)
    for mt in range(4):
        bv[:, mt] = (b_ih + b_hh)[mt * 128:(mt + 1) * 128]
    for i in range(2):
        bv[:, 4 + i] = b_ih[512 + i * 128:512 + (i + 1) * 128]
        bv[:, 6 + i] = b_hh[512 + i * 128:512 + (i + 1) * 128]

    wd2 = np.zeros((8, 8), bf16)
    for c in range(CH):
        for i in range(2):
            for j in range(2):
                wd2[2 * c + j, 2 * c + i] = bf16(_C_EXACT)

    bpk = np.zeros((8, 1), np.float32)
    for c in range(CH):
        bpk[2 * c + 0, 0] = b_out[0]
        bpk[2 * c + 1, 0] = b_out[1]

    init_h = np.asarray(inputs["init_h"], np.float32)
    ctx_in = np.asarray(inputs["ctx"], np.float32)
    x0 = np.asarray(inputs["x0"], np.float32)
    y0 = np.asarray(inputs["y0"], np.float32)

    in_maps = []
    for core in range(N_CORES):
        sl = slice(core * n_sh, (core + 1) * n_sh)
        h0b = np.ascontiguousarray(init_h[sl].T.reshape(2, 128, nloc)).astype(bf16)
        x0i = np.zeros((8, CH, 512), bf16)
        x0i[2:7] = ctx_in[sl].T.reshape(5, CH, 512).astype(bf16)
        pos0 = np.zeros((8, 512), np.float32)
        for c in range(CH):
            pos0[2 * c + 0] = x0[sl].reshape(CH, 512)[c]
            pos0[2 * c + 1] = y0[sl].reshape(CH, 512)[c]
        in_maps.append({
            "h0b": h0b, "x0i": x0i, "pos0": pos0, "wh": wh,
            "wt": wt, "wo": wo, "wd2": wd2, "bv": bv, "bpk": bpk,
        })
    return in_maps


def _host_unpack(results, T, nloc):
    CH = nloc // 512
    outs = []
    for r in results:
        arr = np.asarray(r["out"][:T], np.float32)  # [T, 2CH, 512]
        a = arr.reshape(T, CH, 2, 512).transpose(1, 3, 0, 2)  # ch, s, T, 2
        outs.append(a.reshape(nloc, T, 2))
    return np.concatenate(outs, axis=0)


def kernel(**inputs) -> np.ndarray:
    T = int(inputs["T"])
    N = inputs["init_h"].shape[0]
    nloc = N // N_CORES
    unroll = 2
    nc = _get_module(T, nloc, unroll)
    in_maps = _host_prep(inputs, nloc)
    res = run_bass_kernel_spmd(nc, in_maps, core_ids=list(range(N_CORES)))
    return _host_unpack(res.results, T, nloc)


# revision 7
# speedup vs baseline: 437.8540x; 1.1350x over previous
"""Trainium2 Bass kernel for nn_Decoder — v2 (bf16 gates/state, short chains).

Same transposed data-parallel layout as v1 (8 cores x 2048 batch cols,
feature-major on chip), with these changes:

- h is stored ONLY in bf16 (no fp32 master + shadow copy): the GRU map is
  contractive and the output tolerance absorbs the extra ~2e-3.
- All gate tensors (r, z, n) and the h-update intermediates are bf16 so
  DVE tensor_tensor ops run in 2x mode and ACT evictions can pack.
- h update is 3 ops via h' = n + z*(h-n): STT(h-n), TT(z*...), TT(n+...),
  pairs split between DVE and Pool.
- clip uses a single Newton iteration (seed err 3.4% -> 0.17%), with the
  bf16-rounding of the norm weight compensated exactly by a constant
  factor folded into the final min(k*y, 1) tensor_scalar.
- matmul issue order keeps PE streaming: per chunk all W_hh passes, then
  the K=7 input tails; all W_out matmuls after every chunk's gates.
- T loop is a hardware For_i (program size O(1) in T); out is fp16
  [t_alloc, 8, 512]; `repeats` wraps the loop for timing runs.
"""

import sys

for _p in ("/opt/trn_rl_repo",):
    if _p not in sys.path:
        sys.path.insert(0, _p)

import numpy as np

import concourse.bass as bass
import concourse.tile as tile
from concourse.bacc import Bacc
from concourse import mybir
from concourse.bass_utils import run_bass_kernel_spmd

F32 = mybir.dt.float32
F16 = mybir.dt.float16
BF16 = mybir.dt.bfloat16
I32 = mybir.dt.int32
AF = mybir.ActivationFunctionType
OP = mybir.AluOpType

HID = 256
CTX_DIM = 5
V_MAX = 10.1415
DT = 0.093
MS = V_MAX * DT  # max_step
N_CORES = 8
MAGIC = 0x5F3759DF - 0x400000  # quake magic adjusted for input u' = -0.5*u

ABLATE: set = set()  # sim-only knobs

# bf16 rounding of the norm-matmul constant, compensated in the final min:
# u_psum = c_bf * |d|^2 where c_bf = bf16(-0.5/MS^2); the Newton result is
# y ~= rsqrt(-0.5 * u_psum) = rsqrt(0.25*|d|^2/MS^2 * (c_bf/c_exact)) ...
# we need s = MS/|d| = rsqrt(|d|^2/MS^2), and the magic-seed pipeline
# computes rsqrt for input u' = -0.5*u with u = |d|^2/MS^2.  With the bf16
# weight the effective u' is scaled by rho = c_bf/c_exact, so y = true/sqrt(rho)
# and s = y * sqrt(rho).
import ml_dtypes as _mld
_C_EXACT = -0.5 / (MS * MS)
_C_BF = float(np.float32(_mld.bfloat16(_C_EXACT)))
_KCOMP = float(np.sqrt(_C_BF / _C_EXACT))


def build_module(T: int, nloc: int, unroll: int = 2, t_alloc: int | None = None,
                 repeats: int = 1):
    """Trace the Bass/Tile module for one core (nloc batch columns)."""
    CH = nloc // 512  # column chunks of 512
    assert nloc % 512 == 0
    if t_alloc is None:
        t_alloc = T

    nc = Bacc()

    # ---- DRAM I/O ----
    h0b_d = nc.dram_tensor("h0b", [2, 128, nloc], BF16, kind="ExternalInput")
    x0_d = nc.dram_tensor("x0i", [8, nloc // 512, 512], BF16, kind="ExternalInput")
    pos0_d = nc.dram_tensor("pos0", [8, 512], F32, kind="ExternalInput")
    wh_d = nc.dram_tensor("wh", [2, 128, 768], BF16, kind="ExternalInput")
    wt_d = nc.dram_tensor("wt", [8, 6, 128], BF16, kind="ExternalInput")
    wo_d = nc.dram_tensor("wo", [128, 4, 2, 8], BF16, kind="ExternalInput")
    wd2_d = nc.dram_tensor("wd2", [8, 8], BF16, kind="ExternalInput")
    bv_d = nc.dram_tensor("bv", [128, 8], F32, kind="ExternalInput")
    bpk_d = nc.dram_tensor("bpk", [8, 1], F32, kind="ExternalInput")
    out_d = nc.dram_tensor("out", [t_alloc, 2 * CH, 512], F16,
                           kind="ExternalOutput")

    with tile.TileContext(nc) as tc:
        import contextlib

        ctx = contextlib.ExitStack()
        with ctx:
            singles = ctx.enter_context(tc.tile_pool(name="singles", bufs=1))
            hb_c = []
            for c in range(CH):
                hb_c.append(singles.tile([128, 2, 512], BF16, tag=f"hb{c}", name=f"hb{c}"))
            x_all = singles.tile([8, CH, 512], BF16, tag="xall", name="xall")
            pos = singles.tile([8, 512], F32, tag="pos", name="pos")
            dbtw = singles.tile([8, 512], F32, tag="dbtw", name="dbtw")
            wh = singles.tile([128, 2, 768], BF16, tag="wh", name="wh")
            wt = singles.tile([8, 6, 128], BF16, tag="wt", name="wt")
            wo = singles.tile([128, 4, 2, 8], BF16, tag="wo", name="wo")
            wd2 = singles.tile([8, 8], BF16, tag="wd2", name="wd2")
            bv = singles.tile([128, 8], F32, tag="bv", name="bv")
            bpk = singles.tile([8, 1], F32, tag="bpk", name="bpk")
            nc.vector.memset(dbtw, 0.0)

            # initial loads
            for c in range(CH):
                cs = slice(c * 512, (c + 1) * 512)
                nc.sync.dma_start(
                    out=hb_c[c],
                    in_=h0b_d[:, :, :].transpose([1, 0, 2])[:, :, cs])
            nc.sync.dma_start(out=x_all, in_=x0_d[:, :, :])
            nc.sync.dma_start(out=pos, in_=pos0_d[:, :])
            nc.sync.dma_start(out=wh, in_=wh_d[:, :, :].transpose([1, 0, 2]))
            nc.sync.dma_start(out=wt, in_=wt_d[:, :, :])
            nc.sync.dma_start(out=wo, in_=wo_d[:, :, :, :])
            nc.sync.dma_start(out=wd2, in_=wd2_d[:, :])
            nc.sync.dma_start(out=bv, in_=bv_d[:, :])
            nc.sync.dma_start(out=bpk, in_=bpk_d[:, :])

            # pools (PSUM: 4 + 2 + 1 + 1 = 8 banks)
            pp1 = ctx.enter_context(tc.tile_pool(name="pp1", bufs=4, space="PSUM"))
            pp2 = ctx.enter_context(tc.tile_pool(name="pp2", bufs=2, space="PSUM"))
            pp3 = ctx.enter_context(tc.tile_pool(name="pp3", bufs=1, space="PSUM"))
            ppd = ctx.enter_context(tc.tile_pool(name="ppd", bufs=1, space="PSUM"))
            sb = ctx.enter_context(tc.tile_pool(name="sb", bufs=3))
            sbs = ctx.enter_context(tc.tile_pool(name="sbs", bufs=3))

            def step(t_idx):
                rz_c, n_c, p1g, p2g, p3g = [], [], {}, {}, {}
                # ---- Phase A: gate matmuls, whh first, tails late ----
                for c in range(CH):
                    hb = hb_c[c]
                    for mt in range(4):
                        p1 = pp1.tile([128, 512], F32, tag="p1", name="p1")
                        ms_ = slice(mt * 128, (mt + 1) * 128)
                        nc.tensor.matmul(p1, wh[:, 0, ms_], hb[:, 0, :],
                                         start=True, stop=False)
                        nc.tensor.matmul(p1, wh[:, 1, ms_], hb[:, 1, :],
                                         start=False, stop=False)
                        p1g[(c, mt)] = p1
                    for i in range(2):
                        p3 = pp3.tile([128, 512], F32, tag="p3", name="p3")
                        ms_ = slice(512 + i * 128, 512 + (i + 1) * 128)
                        nc.tensor.matmul(p3, wh[:, 0, ms_], hb[:, 0, :],
                                         start=True, stop=False)
                        nc.tensor.matmul(p3, wh[:, 1, ms_], hb[:, 1, :],
                                         start=False, stop=True)
                        p3g[(c, i)] = p3
                    # K=7 input tails (need x from previous step's clip)
                    for mt in range(4):
                        nc.tensor.matmul(p1g[(c, mt)], wt[0:7, mt, :],
                                         x_all[0:7, c, :], start=False, stop=True)
                    for i in range(2):
                        p2 = pp2.tile([128, 512], F32, tag="p2", name="p2")
                        nc.tensor.matmul(p2, wt[0:7, 4 + i, :], x_all[0:7, c, :],
                                         start=True, stop=True)
                        p2g[(c, i)] = p2

                    # ---- gate evictions + n preact (bf16 outputs) ----
                    rzs = sb.tile([128, 4, 512], BF16, tag="rzs", name="rzs")
                    for mt in range(4):
                        nc.scalar.activation(rzs[:, mt, :], p1g[(c, mt)],
                                             AF.Sigmoid, bias=bv[:, mt:mt + 1])
                    n_t = sb.tile([128, 2, 512], BF16, tag="n", name="n")
                    for i in range(2):
                        t1 = sbs.tile([128, 512], BF16, tag="t1", name="t1")
                        nc.vector.scalar_tensor_tensor(
                            t1, p3g[(c, i)], bv[:, 6 + i:7 + i], rzs[:, i, :],
                            op0=OP.add, op1=OP.mult)
                        npre = sbs.tile([128, 512], F32, tag="npre", name="npre")
                        nc.vector.scalar_tensor_tensor(
                            npre, p2g[(c, i)], bv[:, 4 + i:5 + i], t1,
                            op0=OP.add, op1=OP.add)
                        nc.scalar.activation(n_t[:, i, :], npre, AF.Tanh)
                    rz_c.append(rzs)
                    n_c.append(n_t)

                    # ---- h' = (1-z)*n + z*h: z-branch (zc, u) runs off the
                    # critical path (needs only z and old h); post-tanh chain
                    # is just nw -> h'.
                    for kt in range(2 if 'hupd' not in ABLATE else 0):
                        zb_eng = (nc.gpsimd if (c + kt) % 2 == 0
                                  else nc.vector)
                        zc = sbs.tile([128, 512], BF16, tag=f"zc{kt}",
                                      name=f"zc{kt}")
                        zb_eng.tensor_scalar(
                            zc, rzs[:, 2 + kt, :], -1.0, 1.0,
                            op0=OP.mult, op1=OP.add)
                        u_t = sbs.tile([128, 512], BF16, tag=f"u{kt}",
                                       name=f"u{kt}")
                        zb_eng.tensor_mul(u_t, rzs[:, 2 + kt, :], hb[:, kt, :])
                        nw = sbs.tile([128, 512], BF16, tag=f"nw{kt}",
                                      name=f"nw{kt}")
                        nc.vector.tensor_mul(nw, n_t[:, kt, :], zc)
                        nc.vector.tensor_add(hb[:, kt, :], nw, u_t)

                # ---- W_out after all gates: deltas into one [8,512] bank ----
                pd = ppd.tile([8, 512], F32, tag="pdu", name="pdu")
                for c in range(CH):
                    nc.tensor.matmul(pd, wo[:, c, 0, :], hb_c[c][:, 0, :],
                                     start=(c == 0), stop=False)
                    nc.tensor.matmul(pd, wo[:, c, 1, :], hb_c[c][:, 1, :],
                                     start=False, stop=(c == CH - 1))
                nc.vector.tensor_scalar(dbtw, pd, bpk, None, op0=OP.add)

                # ---- clip: s = min(k*rsqrt_1newton(u'), 1) ----
                if 'clip' in ABLATE:
                    nc.gpsimd.tensor_add(pos, pos, dbtw)
                    pos16 = sbs.tile([8, 512], F16, tag="pos16", name="pos16")
                    nc.gpsimd.tensor_copy(pos16, pos)
                    nc.sync.dma_start(out=out_d[t_idx, :, :], in_=pos16)
                    return
                sqv = sbs.tile([8, 512], BF16, tag="sqv", name="sqv")
                nc.gpsimd.tensor_mul(sqv, dbtw, dbtw)
                pu = ppd.tile([8, 512], F32, tag="pdu", name="pu")
                nc.tensor.matmul(pu, wd2, sqv, start=True, stop=True)
                s1i = sbs.tile([8, 512], I32, tag="s1i", name="s1i")
                nc.vector.tensor_scalar(
                    s1i, pu.bitcast(I32), 1, 0x3FFFFFFF,
                    op0=OP.logical_shift_right, op1=OP.bitwise_and)
                y0i = sbs.tile([8, 512], I32, tag="y0i", name="y0i")
                nc.vector.tensor_scalar(
                    y0i, s1i, MAGIC, -1, op0=OP.subtract, op1=OP.mult)
                y0 = y0i.bitcast(F32)
                m_t = sbs.tile([8, 512], F32, tag="m", name="m")
                nc.vector.tensor_mul(m_t, y0, y0)
                m2_t = sbs.tile([8, 512], F32, tag="m2", name="m2")
                nc.vector.tensor_mul(m2_t, m_t, pu)
                y2_t = sbs.tile([8, 512], F32, tag="y2", name="y2")
                nc.vector.scalar_tensor_tensor(
                    y2_t, m2_t, 1.5, y0, op0=OP.add, op1=OP.mult)
                smin = sbs.tile([8, 512], F32, tag="smin", name="smin")
                nc.vector.tensor_scalar(
                    smin, y2_t, _KCOMP, 1.0, op0=OP.mult, op1=OP.min)
                # x feedback (bf16) for next step's tails: one op + DMAs
                xprod = sbs.tile([8, 512], BF16, tag="xprod", name="xprod")
                nc.vector.tensor_mul(xprod, smin, dbtw)
                for c in range(CH):
                    nc.sync.dma_start(out=x_all[0:2, c, :],
                                      in_=xprod[2 * c:2 * c + 2, :])
                # pos path (off the recurrence): fp32 accumulate, fp16 out
                dct = sbs.tile([8, 512], F32, tag="dct", name="dct")
                nc.gpsimd.tensor_mul(dct, smin, dbtw)
                nc.gpsimd.tensor_add(pos, pos, dct)
                pos16 = sbs.tile([8, 512], F16, tag="pos16", name="pos16")
                nc.gpsimd.tensor_copy(pos16, pos)
                nc.sync.dma_start(out=out_d[t_idx, :, :], in_=pos16)

            if unroll == 0:  # static python loop (TimelineSim / debug)
                for t in range(T):
                    step(t)
            else:
                assert T % unroll == 0
                if repeats == 1:
                    with tc.For_i(0, T, unroll) as iv:
                        for j in range(unroll):
                            step(iv + j)
                else:
                    with tc.For_i(0, repeats, 1):
                        with tc.For_i(0, T, unroll) as iv:
                            for j in range(unroll):
                                step(iv + j)

    nc.finalize()
    return nc


# ---------------- host side ----------------

_module_cache: dict = {}


def _get_module(T: int, nloc: int, unroll: int, t_alloc: int | None = None,
                repeats: int = 1):
    key = (T, nloc, unroll, t_alloc, repeats)
    if key not in _module_cache:
        _module_cache[key] = build_module(T, nloc, unroll, t_alloc, repeats)
    return _module_cache[key]


def _host_prep(inputs, nloc):
    """Build per-core in_maps from full inputs."""
    N = inputs["init_h"].shape[0]
    n_sh = N // N_CORES
    CH = nloc // 512
    W_ih = np.asarray(inputs["W_ih"], np.float32)
    W_hh = np.asarray(inputs["W_hh"], np.float32)
    b_ih = np.asarray(inputs["b_ih"], np.float32)
    b_hh = np.asarray(inputs["b_hh"], np.float32)
    W_out = np.asarray(inputs["W_out"], np.float32)
    b_out = np.asarray(inputs["b_out"], np.float32)

    import ml_dtypes
    bf16 = ml_dtypes.bfloat16
    wh = np.ascontiguousarray(W_hh.T.reshape(2, 128, 768)).astype(bf16)
    woT = W_out.T.reshape(2, 128, 2)  # [kt, 128, i]
    wo = np.zeros((128, 4, 2, 8), np.float32)
    for c in range(4):
        for kt in range(2):
            for i in range(2):
                wo[:, c, kt, 2 * c + i] = woT[kt, :, i]
    wo = wo.astype(bf16)

    # K=7 input tails: rows 0-1 = delta cols of W_ih, rows 2-6 = ctx cols
    wt = np.zeros((8, 6, 128), bf16)
    for mt in range(6):
        if mt < 4:
            rows = slice(mt * 128, (mt + 1) * 128)
        else:
            rows = slice(512 + (mt - 4) * 128, 512 + (mt - 3) * 128)
        wt[0:7, mt, :] = W_ih[rows, :].T.astype(bf16)

    # biases: cols 0-3 = (b_ih+b_hh) rz tiles, 4-5 = b_ih n, 6-7 = b_hh n
    bv = np.zeros((128, 8), np.float32)
    for mt in range(4):
        bv[:, mt] = (b_ih + b_hh)[mt * 128:(mt + 1) * 128]
    for i in range(2):
        bv[:, 4 + i] = b_ih[512 + i * 128:512 + (i + 1) * 128]
        bv[:, 6 + i] = b_hh[512 + i * 128:512 + (i + 1) * 128]

    wd2 = np.zeros((8, 8), bf16)
    for c in range(CH):
        for i in range(2):
            for j in range(2):
                wd2[2 * c + j, 2 * c + i] = bf16(_C_EXACT)

    bpk = np.zeros((8, 1), np.float32)
    for c in range(CH):
        bpk[2 * c + 0, 0] = b_out[0]
        bpk[2 * c + 1, 0] = b_out[1]

    init_h = np.asarray(inputs["init_h"], np.float32)
    ctx_in = np.asarray(inputs["ctx"], np.float32)
    x0 = np.asarray(inputs["x0"], np.float32)
    y0 = np.asarray(inputs["y0"], np.float32)

    in_maps = []
    for core in range(N_CORES):
        sl = slice(core * n_sh, (core + 1) * n_sh)
        h0b = np.ascontiguousarray(init_h[sl].T.reshape(2, 128, nloc)).astype(bf16)
        x0i = np.zeros((8, CH, 512), bf16)
        x0i[2:7] = ctx_in[sl].T.reshape(5, CH, 512).astype(bf16)
        pos0 = np.zeros((8, 512), np.float32)
        for c in range(CH):
            pos0[2 * c + 0] = x0[sl].reshape(CH, 512)[c]
            pos0[2 * c + 1] = y0[sl].reshape(CH, 512)[c]
        in_maps.append({
            "h0b": h0b, "x0i": x0i, "pos0": pos0, "wh": wh,
            "wt": wt, "wo": wo, "wd2": wd2, "bv": bv, "bpk": bpk,
        })
    return in_maps


def _host_unpack(results, T, nloc):
    CH = nloc // 512
    outs = []
    for r in results:
        arr = np.asarray(r["out"][:T], np.float32)  # [T, 2CH, 512]
        a = arr.reshape(T, CH, 2, 512).transpose(1, 3, 0, 2)  # ch, s, T, 2
        outs.append(a.reshape(nloc, T, 2))
    return np.concatenate(outs, axis=0)


def kernel(**inputs) -> np.ndarray:
    T = int(inputs["T"])
    N = inputs["init_h"].shape[0]
    nloc = N // N_CORES
    unroll = 2
    nc = _get_module(T, nloc, unroll)
    in_maps = _host_prep(inputs, nloc)
    res = run_bass_kernel_spmd(nc, in_maps, core_ids=list(range(N_CORES)))
    return _host_unpack(res.results, T, nloc)
